# revision 24
# baseline (speedup 1.0000x reference)
"""Multi-head attention TRN2 Bass kernel, head-sharded across 8 NeuronCores.

Problem: S=2048, E=1024, H=16 heads, dk=dv=64, fp32.
    Q = x @ Wq.T ; K = x @ Wk.T ; V = x @ Wv.T   (per-head slices)
    A_h = softmax(Q_h K_h^T / 8) V_h
    out = concat_h(A_h) @ Wo.T
Sharding: tensor-parallel over heads; core i owns heads (2i, 2i+1) and a
128-column slice of Wo. The 8 partial [2048,1024] outputs are summed on host.

Engine budget per core (the design drivers):
  - ACT exp is 2*S*S = 8.4M elems at 1 elem/lane/cycle @1.2GHz -> ~73us busy
    minimum (64 instrs of [128,1024]). It must start as early as possible and
    never starve.
  - PE: projections are full-K (E=1024) GEMMs; scores contract over dk=64
    only, so they run as two concurrent 64-row tile_position matmuls
    ((0,0)/(64,0)) -> both heads' scores in one 512-col stream span.
  - V is computed weight-stationary as V^T (8 LDWEIGHTS instead of 128) and
    DMA-transposed through the XBAR into the [sk, dv] layout AV needs.

Emission order = per-engine queue order, so the loop is software-pipelined:
  x arrives in per-sequence-block pieces; K/Q/V projections of block t+1 are
  interleaved into block 0's attention chunks (one of K/Q/V per chunk slot);
  scores(c+1) is emitted before AV(c) so the PE streams scores while ACT
  does exp(c); outproj of block b-1 is emitted inside block b's chunk loop.

Softmax normalization rides the AV matmul as ones-columns in the stationary
operand (rows 64/65 of the [66,512] PSUM accumulator collect the exp row
sums); normalization is a reciprocal+broadcast multiply on DVE/GPSIMD.

All matmul operands bf16 (fp32 PSUM accumulation; ~0.5% rel err).
"""

import numpy as np
import ml_dtypes

import concourse.mybir as mybir
import concourse.tile as tile
from concourse import bacc
from concourse.bass_utils import run_bass_kernel_spmd

S, E, H, DK, DV = 2048, 1024, 16, 64, 64
NCORES = 8
HPC = H // NCORES          # heads per core = 2
CSL = HPC * DV             # concat-dim columns per core = 128
P = 128
NE = E // P                # 8 contraction chunks for projections
SQB = 512                  # sequence block (PSUM-bank-limited matmul width)
NSQB = S // SQB            # 4
NCH = S // P               # 16 sk chunks of 128
F32 = mybir.dt.float32
BF16 = mybir.dt.bfloat16
SCALE = 1.0 / np.sqrt(DK).astype(np.float32)  # 1/8

EXP = mybir.ActivationFunctionType.Exp
MULT = mybir.AluOpType.mult

_cache = {}
last_results = None  # BassKernelResults of the most recent run (for test.py)
TRACE = False
DEBUG = False


def _build_nc():
    nc = bacc.Bacc("TRN2", target_bir_lowering=False, debug=False)

    # host pre-arranges everything partition-major (and bf16) for fast DMA
    xT = nc.dram_tensor("xT", [P, NE, S], BF16, kind="ExternalInput")
    wqT = nc.dram_tensor("wqT", [P, NE, CSL], BF16, kind="ExternalInput")
    wkT = nc.dram_tensor("wkT", [P, NE, CSL], BF16, kind="ExternalInput")
    wvT = nc.dram_tensor("wvT", [P, NE, CSL], BF16, kind="ExternalInput")
    woT = nc.dram_tensor("woT", [CSL, E], BF16, kind="ExternalInput")
    y = nc.dram_tensor("y", [S, E], BF16, kind="ExternalOutput")

    dbg = {}
    if DEBUG:
        for nm, shp, dt in (
            ("dbg_q", [P, S], BF16), ("dbg_k", [P, S], BF16),
            ("dbg_va", [P, NCH, DV + 2], BF16), ("dbg_es", [P, 2 * SQB], BF16),
            ("dbg_at", [DV, SQB], F32), ("dbg_den", [1, SQB], F32),
            ("dbg_rsr", [1, SQB], F32),
            ("dbg_bc", [DV, SQB], F32), ("dbg_a1t", [P, SQB], BF16),
        ):
            dbg[nm] = nc.dram_tensor(nm, shp, dt, kind="ExternalOutput")

    xT_r = xT.ap()
    w_r = {"q": wqT.ap(), "k": wkT.ap(), "v": wvT.ap()}
    y_ap = y.ap()

    with tile.TileContext(nc) as tc:
        with tc.tile_pool(name="persist", bufs=1) as persist, \
             tc.tile_pool(name="xw", bufs=1) as xw, \
             tc.tile_pool(name="proj_ps", bufs=2, space="PSUM") as proj_ps, \
             tc.tile_pool(name="sc_ps", bufs=2, space="PSUM") as sc_ps, \
             tc.tile_pool(name="at_ps", bufs=2, space="PSUM") as at_ps, \
             tc.tile_pool(name="est", bufs=6) as est_pool, \
             tc.tile_pool(name="a1t", bufs=2) as a1t_pool, \
             tc.tile_pool(name="small", bufs=8) as small, \
             tc.tile_pool(name="outp", bufs=4) as outp:

            # Persistent SBUF tensors. qt/kt: rows 0-63 head A (dk), 64-127
            # head B.  vtsb: V^T in the same layout.  vaug[h]: V chunks in
            # [sk, dv] + 2 ones columns (softmax denominator rows).
            qt = persist.tile([P, S], BF16)
            # zero-padded per-head K^T so scores stay K=128 full-array mode
            # (mixing 64-row tile_position scores with 128-row AV matmuls
            # mode-switches the PE twice per chunk; measured NaN on HW).
            kpad = [
                persist.tile([P, S], BF16, name=f"kpad{h}", tag=f"kpad{h}")
                for h in range(HPC)
            ]
            vtsb = persist.tile([P, S], BF16)
            vaug = [
                persist.tile([P, NCH, DV + 2], BF16, name=f"vaug{h}", tag=f"vaug{h}")
                for h in range(HPC)
            ]
            # contiguous XBAR-transpose landing zone: dma_start_transpose on a
            # gapped destination (vaug's 66-stride) is wrong on HW, so land
            # in a dense [P, NCH, DV] tile and gpsimd-copy into vaug.
            vstage = [
                persist.tile([P, NCH, DV], BF16, name=f"vstage{h}", tag=f"vstage{h}")
                for h in range(HPC)
            ]
            wosb = persist.tile([P, E], BF16)

            for h in range(HPC):
                nc.gpsimd.memset(vaug[h][:, :, DV : DV + 2], 1.0)
            nc.gpsimd.memset(kpad[0][DK:P, :], 0.0)
            nc.gpsimd.memset(kpad[1][0:DK, :], 0.0)

            # Weights on sync queue first (small, needed by proj t0).
            nc.sync.dma_start(wosb[:], woT.ap())
            wsb = {}
            for m in ("k", "q", "v"):
                wsb[m] = xw.tile([P, NE, CSL], BF16, name=f"w{m}sb", tag=f"w{m}")
                nc.sync.dma_start(wsb[m][:], w_r[m][:])
            # x in per-(t, n) pieces, t-major so block 0 lands first.
            # t0 on sync (front of queue), rest on gpsimd.
            xsb = xw.tile([P, NE, S], BF16)
            for t in range(NSQB):
                tsl = slice(t * SQB, (t + 1) * SQB)
                q = nc.sync if t == 0 else nc.gpsimd
                for n in range(NE):
                    q.dma_start(xsb[:, n, tsl], xT_r[:, n, tsl])

            def emit_proj(m, t, dst):
                """One 512-col block of a projection, weight-stationary,
                accumulated over the 8 E-chunks; cast into dst (bf16).
                dst=None means K: split-cast into the two padded K^T tiles."""
                tsl = slice(t * SQB, (t + 1) * SQB)
                ps = proj_ps.tile([P, SQB], F32, tag="proj")
                for n in range(NE):
                    nc.tensor.matmul(
                        ps[:], lhsT=wsb[m][:, n, :], rhs=xsb[:, n, tsl],
                        start=(n == 0), stop=(n == NE - 1),
                    )
                if dst is None:
                    nc.vector.tensor_copy(kpad[0][0:DK, tsl], ps[0:DK, :])
                    nc.vector.tensor_copy(kpad[1][DK:P, tsl], ps[DK:P, :])
                else:
                    nc.vector.tensor_copy(dst[:, tsl], ps[:])

            def emit_vtrans(t):
                """XBAR-transpose V^T block t into vstage[h][:, 4t:4t+4, :].
                out[p, c, j] = vtsb[64h+j, 512t + 128c + p] = V_h[sk, j];
                then copy into the 66-stride vaug layout on gpsimd."""
                csl4 = slice(4 * t, 4 * t + 4)
                for h in range(HPC):
                    nc.sync.dma_start_transpose(
                        vstage[h][:, csl4, :],
                        vtsb[64 * h : 64 * h + 64, t * SQB : (t + 1) * SQB],
                    )
                    nc.gpsimd.tensor_copy(
                        vaug[h][:, csl4, 0:DV], vstage[h][:, csl4, :]
                    )

            # t0 projections up front (block 0 attention depends on them).
            emit_proj("k", 0, None)
            emit_proj("q", 0, qt)
            emit_proj("v", 0, vtsb)
            emit_vtrans(0)

            def emit_scores(b, c):
                """Both heads' scores^T chunk; zero-padded K keeps the PE in
                a single 128x128 tile mode."""
                bsl = slice(b * SQB, (b + 1) * SQB)
                csl = slice(c * P, (c + 1) * P)
                sc = sc_ps.tile([P, 2 * SQB], F32, tag="sc")
                for h in range(HPC):
                    nc.tensor.matmul(
                        sc[:, h * SQB : (h + 1) * SQB],
                        lhsT=kpad[h][:, csl], rhs=qt[:, bsl],
                        start=True, stop=True,
                    )
                return sc

            def emit_outproj(b, a1t):
                for j in range(NSQB):
                    rsl = slice(b * SQB + j * P, b * SQB + (j + 1) * P)
                    osb = outp.tile([P, E], BF16, tag="osb")
                    for e2 in range(E // SQB):
                        esl = slice(e2 * SQB, (e2 + 1) * SQB)
                        ops = proj_ps.tile([P, SQB], F32, tag="proj")
                        nc.tensor.matmul(
                            ops[:], lhsT=a1t[:, j * P : (j + 1) * P],
                            rhs=wosb[:, esl], start=True, stop=True,
                        )
                        nc.vector.tensor_copy(osb[:, esl], ops[:])
                    (nc.sync if j % 2 else nc.gpsimd).dma_start(y_ap[rsl, :], osb[:])

            def emit_normalize(at, a1t, dump=False):
                """a1t rows = A^T * (1/rowsum).  Bulk-copy the PSUM
                accumulators to SBUF first so the at banks free up fast
                (next block's AV reuses them); head B is shifted to rows
                64-127 via gpsimd sbuf->sbuf DMA (lane-aligned ops can't
                cross partitions)."""
                atsb, dens = [], []
                for h in range(HPC):
                    a = small.tile([DV, SQB], F32, name=f"atsb{h}", tag="atsb")
                    nc.vector.tensor_copy(a[:], at[h][0:DV, :])
                    # denominator row to a partition-0 tile: custom DVE ops
                    # (reciprocal_approx_fast) mis-read partition-offset
                    # single-row inputs on HW; tensor_copy handles it.
                    d = small.tile([1, SQB], F32, name=f"den{h}", tag="den")
                    nc.vector.tensor_copy(d[:], at[h][DV : DV + 1, :])
                    atsb.append(a)
                    dens.append(d)
                if dump:
                    nc.sync.dma_start(dbg["dbg_at"].ap(), atsb[0][:])
                    nc.sync.dma_start(dbg["dbg_den"].ap(), dens[0][:])
                for h in range(HPC):
                    rsr = small.tile([1, SQB], F32, tag="rsr")
                    nc.vector.reciprocal_approx_fast(rsr[:], dens[h][:])
                    bc = small.tile([DV, SQB], F32, tag="bc")
                    nc.gpsimd.partition_broadcast(bc[:], rsr[:])
                    if dump and h == 0:
                        nc.sync.dma_start(dbg["dbg_rsr"].ap(), rsr[:])
                        nc.sync.dma_start(dbg["dbg_bc"].ap(), bc[:])
                    if h == 0:
                        nc.vector.tensor_tensor(
                            a1t[0:DV, :], atsb[h][0:DV, :], bc[:], MULT
                        )
                    else:
                        tb = small.tile([DV, SQB], BF16, tag="tb")
                        nc.vector.tensor_tensor(
                            tb[:], atsb[h][0:DV, :], bc[:], MULT
                        )
                        nc.gpsimd.dma_start(a1t[DV:P, :], tb[:])

            prev_a1t = None
            for b in range(NSQB):
                at = [
                    at_ps.tile([P, SQB], F32, name=f"at{h}", tag="at")
                    for h in range(HPC)
                ]
                a1t = a1t_pool.tile([P, SQB], BF16, tag="a1t")
                sc = emit_scores(b, 0)
                for c in range(NCH):
                    es = est_pool.tile([P, 2 * SQB], BF16, tag="est")
                    nc.scalar.activation(es[:], sc[:], EXP, scale=float(SCALE))
                    if DEBUG and b == 0 and c == 0:
                        nc.sync.dma_start(dbg["dbg_es"].ap(), es[:])
                    # interleave non-ACT-critical PE work behind the exp:
                    if b == 0 and c < 12:
                        t, r = c // 4 + 1, c % 4
                        if r == 0:
                            emit_proj("k", t, None)
                        elif r == 1:
                            emit_proj("q", t, qt)
                        elif r == 2:
                            emit_proj("v", t, vtsb)
                            emit_vtrans(t)
                    if b > 0 and c == 2:
                        emit_outproj(b - 1, prev_a1t)
                    if c < NCH - 1:
                        sc = emit_scores(b, c + 1)
                    for h in range(HPC):
                        nc.tensor.matmul(
                            at[h][0 : DV + 2, :],
                            lhsT=vaug[h][:, c, :],
                            rhs=es[:, h * SQB : (h + 1) * SQB],
                            start=(c == 0), stop=(c == NCH - 1),
                        )
                emit_normalize(at, a1t, dump=(DEBUG and b == 0))
                if DEBUG and b == 0:
                    nc.sync.dma_start(dbg["dbg_a1t"].ap(), a1t[:])
                prev_a1t = a1t
            emit_outproj(NSQB - 1, prev_a1t)

            if DEBUG:
                nc.sync.dma_start(dbg["dbg_q"].ap(), qt[:])
                nc.sync.dma_start(dbg["dbg_k"].ap(), kpad[0][:])
                nc.sync.dma_start(dbg["dbg_va"].ap(), vaug[0][:])

    nc.compile()
    return nc


def kernel(x, Wq, Wk, Wv, Wo):
    global last_results
    x = np.asarray(x, dtype=np.float32)
    Wq = np.asarray(Wq, dtype=np.float32)
    Wk = np.asarray(Wk, dtype=np.float32)
    Wv = np.asarray(Wv, dtype=np.float32)
    Wo = np.asarray(Wo, dtype=np.float32)

    if "nc" not in _cache:
        _cache["nc"] = _build_nc()
    nc = _cache["nc"]

    bf = ml_dtypes.bfloat16
    # [E, S] -> [P, NE, S] partition-major (chunk n, partition p = row n*P+p)
    xT = np.ascontiguousarray(
        x.T.reshape(NE, P, S).transpose(1, 0, 2).astype(bf)
    )
    WqT = np.ascontiguousarray(Wq.T)
    WkT = np.ascontiguousarray(Wk.T)
    WvT = np.ascontiguousarray(Wv.T)
    WoT = np.ascontiguousarray(Wo.T)

    in_maps = []
    for i in range(NCORES):
        sl = slice(i * CSL, (i + 1) * CSL)

        def wslice(WT):
            # [E, CSL] slice -> [P, NE, CSL] partition-major
            return np.ascontiguousarray(
                WT[:, sl].reshape(NE, P, CSL).transpose(1, 0, 2).astype(bf)
            )

        in_maps.append({
            "xT": xT,
            "wqT": wslice(WqT),
            "wkT": wslice(WkT),
            "wvT": wslice(WvT),
            "woT": np.ascontiguousarray(WoT[sl, :].astype(bf)),
        })

    last_results = run_bass_kernel_spmd(
        nc, in_maps, core_ids=list(range(NCORES)), trace=TRACE
    )
    out = np.zeros((S, E), dtype=np.float32)
    for r in last_results.results:
        out += r["y"].astype(np.float32)
    return out


# revision 28
# speedup vs baseline: 1.0469x; 1.0469x over previous
"""Multi-head attention TRN2 Bass kernel, head-sharded across 8 NeuronCores.

Problem: S=2048, E=1024, H=16 heads, dk=dv=64, fp32.
    Q = x @ Wq.T ; K = x @ Wk.T ; V = x @ Wv.T   (per-head slices)
    A_h = softmax(Q_h K_h^T / 8) V_h
    out = concat_h(A_h) @ Wo.T
Sharding: tensor-parallel over heads; core i owns heads (2i, 2i+1) and a
128-column slice of Wo. The 8 partial [2048,1024] outputs are summed on host.

Engine budget per core (the design drivers):
  - ACT exp is 2*S*S = 8.4M elems at 1 elem/lane/cycle @1.2GHz -> ~73us busy
    minimum (64 instrs of [128,1024]). It must start as early as possible and
    never starve.
  - PE: projections are full-K (E=1024) GEMMs; scores contract over dk=64
    only, so they run as two concurrent 64-row tile_position matmuls
    ((0,0)/(64,0)) -> both heads' scores in one 512-col stream span.
  - V is computed weight-stationary as V^T (8 LDWEIGHTS instead of 128) and
    DMA-transposed through the XBAR into the [sk, dv] layout AV needs.

Emission order = per-engine queue order, so the loop is software-pipelined:
  x arrives in per-sequence-block pieces; K/Q/V projections of block t+1 are
  interleaved into block 0's attention chunks (one of K/Q/V per chunk slot);
  scores(c+1) is emitted before AV(c) so the PE streams scores while ACT
  does exp(c); outproj of block b-1 is emitted inside block b's chunk loop.

Softmax normalization rides the AV matmul as ones-columns in the stationary
operand (rows 64/65 of the [66,512] PSUM accumulator collect the exp row
sums); normalization is a reciprocal+broadcast multiply on DVE/GPSIMD.

All matmul operands bf16 (fp32 PSUM accumulation; ~0.5% rel err).
"""

import numpy as np
import ml_dtypes

import concourse.mybir as mybir
import concourse.tile as tile
from concourse import bacc
from concourse.bass_utils import run_bass_kernel_spmd

S, E, H, DK, DV = 2048, 1024, 16, 64, 64
NCORES = 8
HPC = H // NCORES          # heads per core = 2
CSL = HPC * DV             # concat-dim columns per core = 128
P = 128
NE = E // P                # 8 contraction chunks for projections
SQB = 512                  # sequence block (PSUM-bank-limited matmul width)
NSQB = S // SQB            # 4
NCH = S // P               # 16 sk chunks of 128
F32 = mybir.dt.float32
BF16 = mybir.dt.bfloat16
SCALE = 1.0 / np.sqrt(DK).astype(np.float32)  # 1/8

EXP = mybir.ActivationFunctionType.Exp
MULT = mybir.AluOpType.mult

_cache = {}
last_results = None  # BassKernelResults of the most recent run (for test.py)
TRACE = False
DEBUG = False


def _build_nc():
    nc = bacc.Bacc("TRN2", target_bir_lowering=False, debug=False)

    # host pre-arranges everything partition-major (and bf16) for fast DMA
    xT = nc.dram_tensor("xT", [P, NE, S], BF16, kind="ExternalInput")
    wqT = nc.dram_tensor("wqT", [P, NE, CSL], BF16, kind="ExternalInput")
    wkT = nc.dram_tensor("wkT", [P, NE, CSL], BF16, kind="ExternalInput")
    wvT = nc.dram_tensor("wvT", [P, NE, CSL], BF16, kind="ExternalInput")
    woT = nc.dram_tensor("woT", [CSL, E], BF16, kind="ExternalInput")
    y = nc.dram_tensor("y", [S, E], BF16, kind="ExternalOutput")

    dbg = {}
    if DEBUG:
        for nm, shp, dt in (
            ("dbg_q", [P, S], BF16), ("dbg_k", [P, S], BF16),
            ("dbg_va", [P, NCH, DV + 2], BF16), ("dbg_es", [P, 2 * SQB], BF16),
            ("dbg_at", [DV, SQB], F32), ("dbg_den", [1, SQB], F32),
            ("dbg_rsr", [1, SQB], F32),
            ("dbg_bc", [DV, SQB], F32), ("dbg_a1t", [P, SQB], BF16),
        ):
            dbg[nm] = nc.dram_tensor(nm, shp, dt, kind="ExternalOutput")

    xT_r = xT.ap()
    w_r = {"q": wqT.ap(), "k": wkT.ap(), "v": wvT.ap()}
    y_ap = y.ap()

    with tile.TileContext(nc) as tc:
        with tc.tile_pool(name="persist", bufs=1) as persist, \
             tc.tile_pool(name="xw", bufs=1) as xw, \
             tc.tile_pool(name="proj_ps", bufs=2, space="PSUM") as proj_ps, \
             tc.tile_pool(name="sc_ps", bufs=2, space="PSUM") as sc_ps, \
             tc.tile_pool(name="at_ps", bufs=2, space="PSUM") as at_ps, \
             tc.tile_pool(name="est", bufs=6) as est_pool, \
             tc.tile_pool(name="a1t", bufs=2) as a1t_pool, \
             tc.tile_pool(name="small", bufs=8) as small, \
             tc.tile_pool(name="outp", bufs=4) as outp:

            # Persistent SBUF tensors. qt/kt: rows 0-63 head A (dk), 64-127
            # head B.  vtsb: V^T in the same layout.  vaug[h]: V chunks in
            # [sk, dv] + 2 ones columns (softmax denominator rows).
            qt = persist.tile([P, S], BF16)
            # zero-padded per-head K^T so scores stay K=128 full-array mode
            # (mixing 64-row tile_position scores with 128-row AV matmuls
            # mode-switches the PE twice per chunk; measured NaN on HW).
            kpad = [
                persist.tile([P, S], BF16, name=f"kpad{h}", tag=f"kpad{h}")
                for h in range(HPC)
            ]
            vtsb = persist.tile([P, S], BF16)
            vaug = [
                persist.tile([P, NCH, DV + 2], BF16, name=f"vaug{h}", tag=f"vaug{h}")
                for h in range(HPC)
            ]
            # contiguous XBAR-transpose landing zone: dma_start_transpose on a
            # gapped destination (vaug's 66-stride) is wrong on HW, so land
            # in a dense [P, NCH, DV] tile and gpsimd-copy into vaug.
            vstage = [
                persist.tile([P, NCH, DV], BF16, name=f"vstage{h}", tag=f"vstage{h}")
                for h in range(HPC)
            ]
            wosb = persist.tile([P, E], BF16)

            for h in range(HPC):
                nc.gpsimd.memset(vaug[h][:, :, DV : DV + 2], 1.0)
            nc.gpsimd.memset(kpad[0][DK:P, :], 0.0)
            nc.gpsimd.memset(kpad[1][0:DK, :], 0.0)

            # Weights on sync queue first (small, needed by proj t0).
            nc.sync.dma_start(wosb[:], woT.ap())
            wsb = {}
            for m in ("k", "q", "v"):
                wsb[m] = xw.tile([P, NE, CSL], BF16, name=f"w{m}sb", tag=f"w{m}")
                nc.sync.dma_start(wsb[m][:], w_r[m][:])
            # x in per-(t, n) pieces, t-major so block 0 lands first.
            # t0 on sync (front of queue), t1 on gpsimd; t2/t3 are emitted
            # later inside block 0's chunk loop so they don't clog the
            # queues ahead of block-0-critical work.
            xsb = xw.tile([P, NE, S], BF16)

            def emit_xdma(t, q):
                tsl = slice(t * SQB, (t + 1) * SQB)
                for n in range(NE):
                    q.dma_start(xsb[:, n, tsl], xT_r[:, n, tsl])

            emit_xdma(0, nc.sync)
            emit_xdma(1, nc.gpsimd)

            def emit_proj(m, t, dst):
                """One 512-col block of a projection, weight-stationary,
                accumulated over the 8 E-chunks; cast into dst (bf16).
                dst=None means K: split-cast into the two padded K^T tiles."""
                tsl = slice(t * SQB, (t + 1) * SQB)
                ps = proj_ps.tile([P, SQB], F32, tag="proj")
                for n in range(NE):
                    nc.tensor.matmul(
                        ps[:], lhsT=wsb[m][:, n, :], rhs=xsb[:, n, tsl],
                        start=(n == 0), stop=(n == NE - 1),
                    )
                if dst is None:
                    nc.vector.tensor_copy(kpad[0][0:DK, tsl], ps[0:DK, :])
                    nc.vector.tensor_copy(kpad[1][DK:P, tsl], ps[DK:P, :])
                else:
                    nc.vector.tensor_copy(dst[:, tsl], ps[:])

            def emit_vtrans(t):
                """XBAR-transpose V^T block t into vstage[h][:, 4t:4t+4, :].
                out[p, c, j] = vtsb[64h+j, 512t + 128c + p] = V_h[sk, j];
                then copy into the 66-stride vaug layout on gpsimd."""
                csl4 = slice(4 * t, 4 * t + 4)
                for h in range(HPC):
                    # t0 gates block-0 AV: split its two transposes across
                    # the hwdge queues (ACT is idle pre-exp anyway).
                    q = nc.scalar if (t == 0 and h == 0) else nc.sync
                    q.dma_start_transpose(
                        vstage[h][:, csl4, :],
                        vtsb[64 * h : 64 * h + 64, t * SQB : (t + 1) * SQB],
                    )
                    nc.vector.tensor_copy(
                        vaug[h][:, csl4, 0:DV], vstage[h][:, csl4, :]
                    )

            # t0 projections up front (block 0 attention depends on them).
            emit_proj("k", 0, None)
            emit_proj("q", 0, qt)
            emit_proj("v", 0, vtsb)
            emit_vtrans(0)

            def emit_scores(b, c):
                """Both heads' scores^T chunk; zero-padded K keeps the PE in
                a single 128x128 tile mode."""
                bsl = slice(b * SQB, (b + 1) * SQB)
                csl = slice(c * P, (c + 1) * P)
                sc = sc_ps.tile([P, 2 * SQB], F32, tag="sc")
                for h in range(HPC):
                    nc.tensor.matmul(
                        sc[:, h * SQB : (h + 1) * SQB],
                        lhsT=kpad[h][:, csl], rhs=qt[:, bsl],
                        start=True, stop=True,
                    )
                return sc

            def emit_outproj(b, a1t):
                for j in range(NSQB):
                    rsl = slice(b * SQB + j * P, b * SQB + (j + 1) * P)
                    osb = outp.tile([P, E], BF16, tag="osb")
                    for e2 in range(E // SQB):
                        esl = slice(e2 * SQB, (e2 + 1) * SQB)
                        ops = proj_ps.tile([P, SQB], F32, tag="proj")
                        nc.tensor.matmul(
                            ops[:], lhsT=a1t[:, j * P : (j + 1) * P],
                            rhs=wosb[:, esl], start=True, stop=True,
                        )
                        nc.vector.tensor_copy(osb[:, esl], ops[:])
                    (nc.sync if j % 2 else nc.gpsimd).dma_start(y_ap[rsl, :], osb[:])

            def emit_normalize(at, a1t, dump=False):
                """a1t rows = A^T * (1/rowsum).  Bulk-copy the PSUM
                accumulators to SBUF first so the at banks free up fast
                (next block's AV reuses them); head B is shifted to rows
                64-127 via gpsimd sbuf->sbuf DMA (lane-aligned ops can't
                cross partitions)."""
                atsb, dens = [], []
                for h in range(HPC):
                    a = small.tile([DV, SQB], F32, name=f"atsb{h}", tag="atsb")
                    nc.vector.tensor_copy(a[:], at[h][0:DV, :])
                    # denominator row to a partition-0 tile: custom DVE ops
                    # (reciprocal_approx_fast) mis-read partition-offset
                    # single-row inputs on HW; tensor_copy handles it.
                    d = small.tile([1, SQB], F32, name=f"den{h}", tag="den")
                    nc.vector.tensor_copy(d[:], at[h][DV : DV + 1, :])
                    atsb.append(a)
                    dens.append(d)
                if dump:
                    nc.sync.dma_start(dbg["dbg_at"].ap(), atsb[0][:])
                    nc.sync.dma_start(dbg["dbg_den"].ap(), dens[0][:])
                for h in range(HPC):
                    rsr = small.tile([1, SQB], F32, tag="rsr")
                    nc.vector.reciprocal_approx_fast(rsr[:], dens[h][:])
                    bc = small.tile([DV, SQB], F32, tag="bc")
                    nc.gpsimd.partition_broadcast(bc[:], rsr[:])
                    if dump and h == 0:
                        nc.sync.dma_start(dbg["dbg_rsr"].ap(), rsr[:])
                        nc.sync.dma_start(dbg["dbg_bc"].ap(), bc[:])
                    if h == 0:
                        nc.vector.tensor_tensor(
                            a1t[0:DV, :], atsb[h][0:DV, :], bc[:], MULT
                        )
                    else:
                        tb = small.tile([DV, SQB], BF16, tag="tb")
                        nc.vector.tensor_tensor(
                            tb[:], atsb[h][0:DV, :], bc[:], MULT
                        )
                        nc.gpsimd.dma_start(a1t[DV:P, :], tb[:])

            prev_a1t = None
            for b in range(NSQB):
                at = [
                    at_ps.tile([P, SQB], F32, name=f"at{h}", tag="at")
                    for h in range(HPC)
                ]
                a1t = a1t_pool.tile([P, SQB], BF16, tag="a1t")
                sc = emit_scores(b, 0)
                for c in range(NCH):
                    es = est_pool.tile([P, 2 * SQB], BF16, tag="est")
                    nc.scalar.activation(es[:], sc[:], EXP, scale=float(SCALE))
                    if DEBUG and b == 0 and c == 0:
                        nc.sync.dma_start(dbg["dbg_es"].ap(), es[:])
                    # interleave non-ACT-critical PE work behind the exp:
                    if b == 0 and c < 12:
                        t, r = c // 4 + 1, c % 4
                        if r == 0:
                            if t < 3:
                                emit_xdma(t + 1, nc.gpsimd)
                            emit_proj("k", t, None)
                        elif r == 1:
                            emit_proj("q", t, qt)
                        elif r == 2:
                            emit_proj("v", t, vtsb)
                            emit_vtrans(t)
                    if b > 0 and c == 2:
                        emit_outproj(b - 1, prev_a1t)
                    if c < NCH - 1:
                        sc = emit_scores(b, c + 1)
                    for h in range(HPC):
                        nc.tensor.matmul(
                            at[h][0 : DV + 2, :],
                            lhsT=vaug[h][:, c, :],
                            rhs=es[:, h * SQB : (h + 1) * SQB],
                            start=(c == 0), stop=(c == NCH - 1),
                        )
                emit_normalize(at, a1t, dump=(DEBUG and b == 0))
                if DEBUG and b == 0:
                    nc.sync.dma_start(dbg["dbg_a1t"].ap(), a1t[:])
                prev_a1t = a1t
            emit_outproj(NSQB - 1, prev_a1t)

            if DEBUG:
                nc.sync.dma_start(dbg["dbg_q"].ap(), qt[:])
                nc.sync.dma_start(dbg["dbg_k"].ap(), kpad[0][:])
                nc.sync.dma_start(dbg["dbg_va"].ap(), vaug[0][:])

    nc.compile()
    return nc


def kernel(x, Wq, Wk, Wv, Wo):
    global last_results
    x = np.asarray(x, dtype=np.float32)
    Wq = np.asarray(Wq, dtype=np.float32)
    Wk = np.asarray(Wk, dtype=np.float32)
    Wv = np.asarray(Wv, dtype=np.float32)
    Wo = np.asarray(Wo, dtype=np.float32)

    if "nc" not in _cache:
        _cache["nc"] = _build_nc()
    nc = _cache["nc"]

    bf = ml_dtypes.bfloat16
    # [E, S] -> [P, NE, S] partition-major (chunk n, partition p = row n*P+p)
    xT = np.ascontiguousarray(
        x.T.reshape(NE, P, S).transpose(1, 0, 2).astype(bf)
    )
    WqT = np.ascontiguousarray(Wq.T)
    WkT = np.ascontiguousarray(Wk.T)
    WvT = np.ascontiguousarray(Wv.T)
    WoT = np.ascontiguousarray(Wo.T)

    in_maps = []
    for i in range(NCORES):
        sl = slice(i * CSL, (i + 1) * CSL)

        def wslice(WT):
            # [E, CSL] slice -> [P, NE, CSL] partition-major
            return np.ascontiguousarray(
                WT[:, sl].reshape(NE, P, CSL).transpose(1, 0, 2).astype(bf)
            )

        in_maps.append({
            "xT": xT,
            "wqT": wslice(WqT),
            "wkT": wslice(WkT),
            "wvT": wslice(WvT),
            "woT": np.ascontiguousarray(WoT[sl, :].astype(bf)),
        })

    last_results = run_bass_kernel_spmd(
        nc, in_maps, core_ids=list(range(NCORES)), trace=TRACE
    )
    out = np.zeros((S, E), dtype=np.float32)
    for r in last_results.results:
        out += r["y"].astype(np.float32)
    return out
